# revision 10
# baseline (speedup 1.0000x reference)
"""Trainium2 Bass kernel for nn_DiffAttention — node-major 4-pass dma_gather.

Math (edge i: src s -> dst n, per-dst softmax over incoming edges):
  p = h @ W_fc.T ; q = p @ w_attn ; w_i = exp(tanh(q_dst[n] - q_src[s]))
  out[n] = elu(p_dst[n] - (sum_i w_i p_src[s_i]) / (sum_i w_i))
(e = tanh(..) in [-1,1] so softmax max-subtraction is unnecessary.)

Device strategy (8 cores, SPMD, dst-sharded 12544 nodes/core):
  - fp16 src table [100352, 128]: rows [p(64)|one|q|junk], node id permuted
    into 4 chunks of 25088 rows (25087 real + 1 zero dummy) so every
    dma_gather idx fits in int16. Built sharded on PE, AllGather'd.
  - 4 passes per core: pass k covers edges with src in chunk k. Nodes are
    re-sorted by pass-degree; groups of <=8 subwindows x 128 nodes share a
    uniform per-node slot count F. One dma_gather per group fetches all
    edge rows node-major: slot j -> partition j%128, block j//128 = (sub,f).
    Per-edge w on ACT (qd is per-partition!), weighted rows by in-place DVE
    mult, per-node sums by DVE reduce along f. Partials [swp|sw] -> fp16
    tables in pass order.
  - Combine: per pass one dma_gather re-orders partials to canonical node
    layout (p=n//98, col=n%98); sum, then batched epilogue
    elu(p_dst - swp/sw) with zero-degree masking; one plain DMA out.
Host does only index prep (degree sorts, idx arrays, permuted h copies).
"""
import sys
sys.path.insert(0, "/opt/trn_rl_repo")
import numpy as np

N = 100000
D = 64
NC = 8
SHARD = 12544            # nodes per core = 128 * 98
COLS = 98
RCH = 25087              # real nodes per chunk
CROWS = 25088            # table rows per chunk (last row zero dummy)
NPAD = CROWS * 4         # 100352
NCHUNK = 4
ELEM = 128               # fp16 elems per src-table row (256B)
MAXSLOT = 32             # max nsub*F per gather group (pay tile 8KB/part)
MAXSUB = 8
MAIN_REPEAT = 1


# ---------------------------------------------------------------- host prep
def _wrap_idx(flat):
    """[n] int -> [128, n//16] int16, idx j at [16s + j%16, j//16] stripes
    replicated (HW SWDGE reads stripe 16:32; interp reads 0:16)."""
    w = flat.reshape(-1, 16).T
    return np.ascontiguousarray(np.tile(w, (8, 1)).astype(np.int16))


def _prep(src, dst):
    src = np.asarray(src, np.int64)
    dst = np.asarray(dst, np.int64)
    if np.any(np.diff(dst) < 0):
        o = np.argsort(dst, kind="stable")
        src, dst = src[o], dst[o]
    per_core = []
    for c in range(NC):
        n_lo = c * SHARD
        e_lo = np.searchsorted(dst, n_lo)
        e_hi = np.searchsorted(dst, min(n_lo + SHARD, N))
        s = src[e_lo:e_hi]
        d = dst[e_lo:e_hi] - n_lo
        passes = []
        for k in range(NCHUNK):
            m = (s // RCH) == k
            sk = (s[m] % RCH).astype(np.int64)
            dk = d[m]
            deg = np.bincount(dk, minlength=SHARD)
            order = np.argsort(-deg, kind="stable")
            rank = np.empty(SHARD, np.int64)
            rank[order] = np.arange(SHARD)
            eo = np.argsort(rank[dk], kind="stable")
            sk = sk[eo]
            cnt = deg[order]
            starts = np.concatenate([[0], np.cumsum(cnt)])
            passes.append(dict(cnt=cnt, starts=starts, sk=sk,
                               order=order, rank=rank))
        per_core.append(passes)

    # cross-core per-subwindow max degree -> shared group schema per pass
    schema = []   # per pass: list of (sw0, nsub, F)
    for k in range(NCHUNK):
        fsub = np.zeros(COLS, np.int64)
        for c in range(NC):
            cnt = per_core[c][k]["cnt"]
            fsub = np.maximum(fsub, cnt[0:SHARD:128])
        groups = []
        sw = 0
        while sw < COLS:
            f_g = int(fsub[sw])
            if f_g == 0:
                groups.append((sw, COLS - sw, 0))
                break
            nsub = 1
            while (sw + nsub < COLS and nsub < MAXSUB
                   and (nsub + 1) * int(fsub[sw]) <= MAXSLOT):
                nsub += 1
            groups.append((sw, nsub, f_g))
            sw += nsub
        schema.append(groups)

    cores = []
    for c in range(NC):
        gidx, qidx, cidx = [], [], []
        for k in range(NCHUNK):
            P = per_core[c][k]
            cols_k = []
            for (sw0, nsub, f_g) in schema[k]:
                if f_g == 0:
                    continue
                ni = 128 * nsub * f_g
                j = np.arange(ni)
                p = j % 128
                b = j // 128
                r = (sw0 + b // f_g) * 128 + p
                f = b % f_g
                idxf = np.full(ni, RCH, np.int64)      # chunk dummy row
                valid = f < P["cnt"][r]
                ei = P["starts"][r] + f
                idxf[valid] = P["sk"][ei[valid]]
                cols_k.append(_wrap_idx(idxf))
            gidx.append(np.concatenate(cols_k, axis=1) if cols_k
                        else np.zeros((128, 0), np.int16))
            qidx.append(_wrap_idx(P["order"]))
            jc = np.arange(SHARD)
            nloc = (jc % 128) * COLS + jc // 128
            rr = P["rank"][nloc]
            cidx.append(_wrap_idx((rr % 128) * COLS + rr // 128))
        cores.append(dict(
            gidx=np.concatenate(gidx, axis=1),
            qidx=np.stack(qidx), cidx=np.stack(cidx)))
    return schema, cores


def _perm_h_src(h):
    hp = np.zeros((NPAD, D + 1), np.float32)
    r = np.arange(NPAD)
    rin = r % CROWS
    n = (r // CROWS) * RCH + rin
    real = (rin < RCH) & (n < N)
    hp[real, :D] = h[n[real]]
    hp[real, D] = 1.0
    return hp


def _local_h_dst(h, c):
    hp = np.zeros((SHARD, D + 1), np.float32)
    n_lo = c * SHARD
    nn = min(SHARD, N - n_lo)
    hp[:nn, :D] = h[n_lo:n_lo + nn]
    hp[:nn, D] = 1.0
    return hp


def _blockT(x):   # [12544, 65] -> [98, 65, 128]
    return np.ascontiguousarray(x.reshape(COLS, 128, D + 1).transpose(0, 2, 1))


def _mkM(W, wat):
    M = np.zeros((D + 1, 66), np.float32)
    M[:D, :D] = W.T
    M[D, D] = 1.0
    M[:D, 65] = W.T @ wat
    return M


# ---------------------------------------------------------------- device
def _build_program(schema, gcols, repeat):
    from concourse import bass, bacc, mybir, tile
    from concourse.library_config import mlp as mlp_lib
    f32, f16, i16 = mybir.dt.float32, mybir.dt.float16, mybir.dt.int16
    FN = mybir.ActivationFunctionType
    OP = mybir.AluOpType

    SLOT = max([MAXSLOT] + [n * f for p in schema for (_, n, f) in p])
    nc = bacc.Bacc("TRN2", target_bir_lowering=False, debug=False,
                   num_devices=NC, num_swdge_queues=4)
    hsT_e = nc.dram_tensor("hsT", [COLS, D + 1, 128], f32,
                           kind="ExternalInput")
    hdT_e = nc.dram_tensor("hdT", [COLS, D + 1, 128], f32,
                           kind="ExternalInput")
    mm_e = nc.dram_tensor("mm", [D + 1, 66], f32, kind="ExternalInput")
    gidx_e = nc.dram_tensor("gidx", [128, gcols], i16, kind="ExternalInput")
    qidx_e = nc.dram_tensor("qidx", [NCHUNK, 128, SHARD // 16], i16,
                            kind="ExternalInput")
    cidx_e = nc.dram_tensor("cidx", [NCHUNK, 128, SHARD // 16], i16,
                            kind="ExternalInput")
    res_e = nc.dram_tensor("res", [SHARD, D], f16, kind="ExternalOutput")

    with tile.TileContext(nc) as tc:
        with tc.tile_pool(name="cst", bufs=1) as cp, \
             tc.tile_pool(name="mn", bufs=1) as sp, \
             tc.tile_pool(name="dr", bufs=1, space="DRAM") as dp:
            bp = tc.alloc_tile_pool(name="bld", bufs=3)
            pp = tc.alloc_tile_pool(name="ps", bufs=2, space="PSUM")
            nc.gpsimd.load_library(mlp_lib)
            mm = cp.tile([D + 1, 66], f32)
            nc.sync.dma_start(out=mm[:], in_=mm_e[:])

            tsrc_sh = dp.tile([SHARD, ELEM], f16)
            tdst = dp.tile([SHARD, D], f16)
            qtab = dp.tile([SHARD, ELEM], f16)
            for b in range(COLS):
                hs = bp.tile([D + 1, 128], f32, tag="hs")
                nc.sync.dma_start(out=hs[:], in_=hsT_e[b])
                ps = pp.tile([128, 66], f32, space="PSUM", tag="ps")
                nc.tensor.matmul(out=ps[:], lhsT=hs[:], rhs=mm[:],
                                 start=True, stop=True)
                t16 = bp.tile([128, ELEM], f16, tag="t16")
                nc.vector.tensor_copy(t16[:, 0:66], ps[:])
                nc.sync.dma_start(out=tsrc_sh[b * 128:(b + 1) * 128, :],
                                  in_=t16[:])
                hd = bp.tile([D + 1, 128], f32, tag="hd")
                nc.sync.dma_start(out=hd[:], in_=hdT_e[b])
                ps2 = pp.tile([128, 66], f32, space="PSUM", tag="ps2")
                nc.tensor.matmul(out=ps2[:], lhsT=hd[:], rhs=mm[:],
                                 start=True, stop=True)
                td = bp.tile([128, D], f16, tag="td")
                nc.vector.tensor_copy(td[:], ps2[:, 0:64])
                nc.sync.dma_start(out=tdst[b * 128:(b + 1) * 128, :],
                                  in_=td[:])
                q16 = bp.tile([128, ELEM], f16, tag="q16")
                nc.scalar.activation(out=q16[:, 0:1], in_=ps2[:, 65:66],
                                     func=FN.Copy)
                nc.sync.dma_start(out=qtab[b * 128:(b + 1) * 128, :],
                                  in_=q16[:])

            tsrc = dp.tile([NPAD, ELEM], f16)
            nc.gpsimd.collective_compute(
                "AllGather", OP.bypass, replica_groups=[list(range(NC))],
                ins=[tsrc_sh.opt()], outs=[tsrc.opt()])

            # qd per pass in rank layout: [128, NCHUNK*COLS] f32
            qd = cp.tile([128, NCHUNK * COLS], f32)
            QSPL = [24, 24, 25, 25]     # col-blocks per sub-gather
            for k in range(NCHUNK):
                qi = bp.tile([128, SHARD // 16], i16, tag="qi")
                nc.sync.dma_start(out=qi[:], in_=qidx_e[k])
                c0 = 0
                for h, nb in enumerate(QSPL):
                    nih = nb * 128
                    qrows = sp.tile([128, SLOT * ELEM], f16, tag="pay",
                                    bufs=8)
                    qv = qrows[:, :nih].rearrange("p (a b) -> p a b", b=ELEM)
                    nc.gpsimd.dma_gather(
                        qv, qtab[:], qi[:, c0 * 8:(c0 + nb) * 8],
                        nih, nih, ELEM, single_packet=False,
                        queue_num=h % 4)
                    nc.vector.tensor_copy(
                        qd[:, k * COLS + c0:k * COLS + c0 + nb],
                        qv[:, :, 0:1].squeeze(2))
                    c0 += nb
            ci = []
            for k in range(NCHUNK):
                t = cp.tile([128, SHARD // 16], i16, tag=f"ci{k}")
                nc.sync.dma_start(out=t[:], in_=cidx_e[k])
                ci.append(t)
            bp.release()

            parts = [dp.tile([128, COLS, ELEM], f16, name=f"part{k}")
                     for k in range(NCHUNK)]

            rep = tc.For_i(0, repeat, 1) if repeat > 1 else None
            if rep is not None:
                rep.__enter__()
            off = 0
            qbytes = [0, 0, 0, 0]
            acc16 = sp.tile([128, COLS, 66], f16, tag="acc16")

            def emit_combine(kc):
                pt = sp.tile([128, COLS * ELEM], f16, tag="pt", bufs=1,
                             name=f"pt{kc}")
                ptv = pt[:].rearrange("p (a b) -> p a b", b=ELEM)
                nc.gpsimd.dma_gather(
                    ptv, parts[kc][:].rearrange("p g e -> (p g) e"),
                    ci[kc][:], SHARD, SHARD, ELEM, single_packet=False,
                    queue_num=kc % 4)
                if kc == 0:
                    nc.vector.tensor_copy(acc16[:], ptv[:, :, 0:66])
                else:
                    nc.vector.tensor_tensor(acc16[:], acc16[:],
                                            ptv[:, :, 0:66], op=OP.add)

            for k in range(NCHUNK):
                pending_combine = k - 1 if k > 0 else None
                for (sw0, nsub, f_g) in schema[k]:
                    if f_g == 0:
                        nz = nsub
                        zc = sw0
                        while nz > 0:
                            zn = min(nz, MAXSUB)
                            zt = sp.tile([128, MAXSUB, ELEM], f16, tag="zt",
                                         bufs=2)
                            nc.vector.memset(zt[:, :zn, :], 0.0)
                            nc.sync.dma_start(
                                out=parts[k][:, zc:zc + zn, :],
                                in_=zt[:, :zn, :])
                            zc += zn
                            nz -= zn
                        continue
                    nig = 128 * nsub * f_g
                    ncol = nig // 16
                    it = sp.tile([128, SLOT * 8], i16, tag="it", bufs=7)
                    nc.sync.dma_start(out=it[:, :ncol],
                                      in_=gidx_e[:, off:off + ncol])
                    off += ncol
                    pay = sp.tile([128, SLOT * ELEM], f16, tag="pay",
                                  bufs=8)
                    pay4 = pay[:, :nsub * f_g * ELEM].rearrange(
                        "p (s f e) -> p s f e", s=nsub, e=ELEM)
                    qsel = min(range(4), key=qbytes.__getitem__)
                    qbytes[qsel] += nig
                    nc.gpsimd.dma_gather(
                        pay[:, :nsub * f_g * ELEM].rearrange(
                            "p (a b) -> p a b", b=ELEM),
                        tsrc[k * CROWS:(k + 1) * CROWS, :],
                        it[:, :ncol], nig, nig, ELEM, single_packet=False,
                        queue_num=qsel)
                    # per-edge weight: w = exp(tanh(qd - qs))
                    dif = sp.tile([128, SLOT], f32, tag="dif", bufs=3)
                    difv = dif[:, :nsub * f_g].rearrange(
                        "p (s f) -> p s f", s=nsub)
                    nc.vector.tensor_tensor(
                        difv,
                        qd[:, k * COLS + sw0:k * COLS + sw0 + nsub]
                        .unsqueeze(2).broadcast_to((128, nsub, f_g)),
                        pay4[:, :, :, 65], op=OP.subtract)
                    th = sp.tile([128, SLOT], f32, tag="th", bufs=3)
                    nc.scalar.activation(out=th[:, :nsub * f_g],
                                         in_=dif[:, :nsub * f_g],
                                         func=FN.Tanh)
                    w16 = sp.tile([128, SLOT], f16, tag="w16", bufs=3)
                    nc.scalar.activation(out=w16[:, :nsub * f_g],
                                         in_=th[:, :nsub * f_g], func=FN.Exp)
                    # weighted rows in place: pay[:,:,:,0:65] *= w
                    payT = pay4[:, :, :, 0:65].transpose([0, 1, 3, 2])
                    nc.vector.tensor_tensor(
                        payT, payT,
                        w16[:, :nsub * f_g].rearrange("p (s f) -> p s f",
                                                      s=nsub)
                        .unsqueeze(2).broadcast_to((128, nsub, 65, f_g)),
                        op=OP.mult)
                    acc = sp.tile([128, MAXSUB * 65], f32, tag="acc", bufs=3)
                    nc.vector.tensor_reduce(
                        out=acc[:, :nsub * 65].rearrange(
                            "p (s e) -> p s e", s=nsub),
                        in_=payT, axis=mybir.AxisListType.X, op=OP.add)
                    pout = sp.tile([128, MAXSUB, ELEM], f16, tag="pout",
                                   bufs=3)
                    nc.scalar.activation(
                        out=pout[:, :nsub, 0:65],
                        in_=acc[:, :nsub * 65].rearrange(
                            "p (s e) -> p s e", s=nsub), func=FN.Copy)
                    nc.sync.dma_start(out=parts[k][:, sw0:sw0 + nsub, :],
                                      in_=pout[:, :nsub, :])
                    if pending_combine is not None:
                        # previous pass's combine: its partials are long
                        # done; one group of this pass is already queued
                        emit_combine(pending_combine)
                        pending_combine = None
                if pending_combine is not None:
                    emit_combine(pending_combine)
            emit_combine(NCHUNK - 1)

            # ---- epilogue (canonical layout: node = p*98 + col)
            pd = sp.tile([128, COLS, D], f16, tag="pd")
            nc.sync.dma_start(
                out=pd[:], in_=tdst[:].rearrange("(p c) e -> p c e", p=128))
            sw_ = acc16[:, :, 64:65].squeeze(2)
            z = sp.tile([128, COLS], f32, tag="z")
            nc.vector.tensor_scalar(out=z[:], in0=sw_, scalar1=0.0,
                                    scalar2=None, op0=OP.is_equal)
            den = sp.tile([128, COLS], f32, tag="den")
            nc.vector.tensor_tensor(den[:], sw_, z[:], op=OP.add)
            rec = sp.tile([128, COLS], f32, tag="rec")
            nc.vector.reciprocal(rec[:], den[:])
            nzm = sp.tile([128, COLS], f32, tag="nzm")
            nc.vector.tensor_scalar(out=nzm[:], in0=z[:], scalar1=-1.0,
                                    scalar2=1.0, op0=OP.mult, op1=OP.add)
            mean = sp.tile([128, COLS, D], f16, tag="e16", bufs=2)
            nc.vector.tensor_tensor(
                mean[:], acc16[:, :, 0:64],
                rec[:].unsqueeze(2).broadcast_to((128, COLS, D)), op=OP.mult)
            df = sp.tile([128, COLS, D], f16, tag="df")
            nc.vector.tensor_tensor(df[:], pd[:], mean[:], op=OP.subtract)
            nc.vector.tensor_tensor(
                df[:], df[:],
                nzm[:].unsqueeze(2).broadcast_to((128, COLS, D)), op=OP.mult)
            ng = sp.tile([128, COLS, D], f16, tag="e16", bufs=2)
            nc.vector.tensor_scalar(out=ng[:], in0=df[:], scalar1=0.0,
                                    scalar2=None, op0=OP.min)
            ex = sp.tile([128, COLS, D], f16, tag="e16", bufs=2)
            nc.scalar.activation(out=ex[:], in_=ng[:], func=FN.Exp)
            nc.vector.tensor_scalar(out=df[:], in0=df[:], scalar1=0.0,
                                    scalar2=None, op0=OP.max)
            resf = sp.tile([128, COLS, D], f16, tag="resf")
            nc.vector.scalar_tensor_tensor(
                out=resf[:], in0=ex[:], scalar=-1.0, in1=df[:],
                op0=OP.add, op1=OP.add)
            nc.sync.dma_start(
                out=res_e[:].rearrange("(p c) e -> p c e", p=128),
                in_=resf[:])
            if rep is not None:
                rep.__exit__(None, None, None)
            pp.release()
    nc.compile()
    return nc


_CACHE = {}


def _get_program(schema, gcols, repeat):
    key = (tuple(tuple(g) for p in schema for g in p), gcols, repeat)
    if key not in _CACHE:
        _CACHE[key] = _build_program(schema, gcols, repeat)
    return _CACHE[key]


def kernel(h_src, h_dst, W_fc, w_attn, src, dst, _main_repeat=MAIN_REPEAT):
    from concourse.bass_utils import run_bass_kernel_spmd

    h_src = np.ascontiguousarray(np.asarray(h_src, np.float32))
    h_dst = np.ascontiguousarray(np.asarray(h_dst, np.float32))
    W_fc = np.ascontiguousarray(np.asarray(W_fc, np.float32))
    w_attn = np.ascontiguousarray(np.asarray(w_attn, np.float32)).reshape(D)
    schema, cores = _prep(src, dst)
    gcols = cores[0]["gidx"].shape[1]

    hsp = _perm_h_src(h_src)
    M = _mkM(W_fc, w_attn)
    in_maps = []
    for c in range(NC):
        in_maps.append({
            "hsT": _blockT(hsp[c * SHARD:(c + 1) * SHARD]),
            "hdT": _blockT(_local_h_dst(h_dst, c)),
            "mm": M,
            "gidx": cores[c]["gidx"],
            "qidx": cores[c]["qidx"],
            "cidx": cores[c]["cidx"],
        })
    nc = _get_program(schema, gcols, _main_repeat)
    res = run_bass_kernel_spmd(nc, in_maps, list(range(NC)))

    out = np.zeros((N, D), np.float32)
    for c in range(NC):
        nn = min(SHARD, N - c * SHARD)
        out[c * SHARD:c * SHARD + nn] = res.results[c]["res"][:nn]
    return out


# ---------------------------------------------------------------- local sim
def simulate(h_src, h_dst, W_fc, w_attn, src, dst):
    """Numpy mirror of the device program (incl. fp16 quantization)."""
    h_src = np.asarray(h_src, np.float32)
    h_dst = np.asarray(h_dst, np.float32)
    W_fc = np.asarray(W_fc, np.float32)
    w_attn = np.asarray(w_attn, np.float32).reshape(D)
    schema, cores = _prep(src, dst)
    M = _mkM(W_fc, w_attn)
    hsp = _perm_h_src(h_src)
    tab16 = (hsp @ M).astype(np.float16)         # [NPAD, 66]
    out = np.zeros((N, D), np.float32)
    for c in range(NC):
        hd = _local_h_dst(h_dst, c)
        pdq = hd @ M                              # [SHARD, 66] f32
        qtab16 = pdq[:, 65].astype(np.float16)
        parts = []
        src64 = np.asarray(src, np.int64)
        dst64 = np.asarray(dst, np.int64)
        P = cores[c]
        # recompute per-pass structures (same as _prep)
        n_lo = c * SHARD
        e_lo = np.searchsorted(dst64, n_lo)
        e_hi = np.searchsorted(dst64, min(n_lo + SHARD, N))
        s_ = src64[e_lo:e_hi]
        d_ = dst64[e_lo:e_hi] - n_lo
        for k in range(NCHUNK):
            part = np.zeros((SHARD, 66), np.float16)   # row = p*98 + G
            m = (s_ // RCH) == k
            deg = np.bincount(d_[m], minlength=SHARD)
            order = np.argsort(-deg, kind="stable")
            qd_rank = qtab16[order].astype(np.float32)   # [rank]
            # decode gidx arrays back? simpler: recompute idxf same way
            sk = (s_[m] % RCH).astype(np.int64)
            rank = np.empty(SHARD, np.int64)
            rank[order] = np.arange(SHARD)
            eo = np.argsort(rank[d_[m]], kind="stable")
            sk = sk[eo]
            cnt = deg[order]
            starts = np.concatenate([[0], np.cumsum(cnt)])
            for (sw0, nsub, f_g) in schema[k]:
                if f_g == 0:
                    continue
                ni = 128 * nsub * f_g
                j = np.arange(ni)
                p = j % 128
                b = j // 128
                r = (sw0 + b // f_g) * 128 + p
                f = b % f_g
                idxf = np.full(ni, RCH, np.int64)
                valid = f < cnt[r]
                idxf[valid] = sk[(starts[r] + f)[valid]]
                rows = tab16[k * CROWS + idxf]            # [ni, 66]
                qs = rows[:, 65].astype(np.float32)
                dif = qd_rank[r] - qs
                w16 = np.exp(np.tanh(dif)).astype(np.float16)
                wp = (rows[:, 0:65] * w16[:, None]).astype(np.float16)
                acc = wp.astype(np.float32).reshape(nsub, f_g, 128, 65) \
                    .sum(axis=1)                           # [nsub? ...]
                # careful: j order is (b=(sub,f), p): reshape [(nsub f) 128]
                part_rows = acc.astype(np.float16)         # [nsub, 128, 65]
                for s2 in range(nsub):
                    G = sw0 + s2
                    part[np.arange(128) * COLS + G, 0:65] = part_rows[s2]
            parts.append(part)
        # combine in canonical layout
        acc16 = np.zeros((SHARD, 66), np.float16)
        for k in range(NCHUNK):
            m = (s_ // RCH) == k
            deg = np.bincount(d_[m], minlength=SHARD)
            rank = np.empty(SHARD, np.int64)
            rank[np.argsort(-deg, kind="stable")] = np.arange(SHARD)
            nloc = np.arange(SHARD)
            rr = rank[nloc]
            rowid = (rr % 128) * COLS + rr // 128
            acc16 = (acc16 + parts[k][rowid]).astype(np.float16)
        swv = acc16[:, 64].astype(np.float32)
        z = (swv == 0.0).astype(np.float32)
        rec = 1.0 / (swv + z)
        nzm = 1.0 - z
        mean = (acc16[:, 0:64].astype(np.float32)
                * rec[:, None]).astype(np.float16).astype(np.float32)
        pd16 = pdq[:, 0:64].astype(np.float16).astype(np.float32)
        df = ((pd16 - mean).astype(np.float16).astype(np.float32)
              * nzm[:, None]).astype(np.float16).astype(np.float32)
        resv = np.where(df > 0, df, np.expm1(np.minimum(df, 0)))
        nn = min(SHARD, N - c * SHARD)
        out[c * SHARD:c * SHARD + nn] = resv[:nn]
    return out


if __name__ == "__main__":
    d = np.load("/root/problem/refdata.npz")
    o = kernel(d["h_src"], d["h_dst"], d["W_fc"], d["w_attn"],
               d["src"], d["dst"])
    exp = d["expected"]
    rel = np.linalg.norm(o - exp) / np.linalg.norm(exp)
    print(f"rel_l2 = {rel:.3e}")
